# revision 1
# baseline (speedup 1.0000x reference)
"""BLSTM Trainium2 kernel: 8-core SPMD.

Sharding: core pair q={2q,2q+1} owns batch element q (41 frames of width 200).
Even core runs the forward 2-layer LSTM chain, odd core the backward chain
(host feeds it time-reversed frames). One pairwise AllGather exchanges the
final-layer hidden streams; each core then projects (Wp), overlap-adds its
half of the time axis, adds the skip connection, and returns a padded
[512, 2300] slice that the host trims and concatenates.
"""
import numpy as np
from contextlib import ExitStack

U = 512
S = 200          # frame width (LSTM steps)
F = 41           # frames per batch element
T = 4200
STRIDE = 100
COLS = S * F     # 8200 time-major columns per core
G = 4 * U        # 2048 gate rows
NCORES = 8
SEG = 2100       # half of T, per-core output segment
PAD = 100
ACC_W = SEG + 2 * PAD  # 2300
KT = U // 128    # 4 k-tiles
MT = G // 128    # 16 m-tiles
FR_LOC = 21      # frames projected per core (1 overlap frame)

_CACHE = {}


def _build():
    import os
    steps = int(os.environ.get("BL_STEPS", S))
    do_coll = os.environ.get("BL_COLLECTIVE", "1") == "1"
    do_dyn = os.environ.get("BL_DYN", "1") == "1"
    do_proj = os.environ.get("BL_PROJ", "1") == "1"
    import concourse.bacc as bacc
    import concourse.tile as tile
    import concourse.bass as bass
    from concourse import mybir

    f32 = mybir.dt.float32
    f16 = mybir.dt.float16
    AF = mybir.ActivationFunctionType

    nc = bacc.Bacc("TRN2", target_bir_lowering=False, debug=False,
                   num_devices=NCORES)

    xT = nc.dram_tensor("xT", [U, COLS], f16, kind="ExternalInput")
    Wx0 = nc.dram_tensor("Wx0", [U, G], f16, kind="ExternalInput")
    Wh0 = nc.dram_tensor("Wh0", [U, G], f16, kind="ExternalInput")
    Wx1 = nc.dram_tensor("Wx1", [U, G], f16, kind="ExternalInput")
    Wh1 = nc.dram_tensor("Wh1", [U, G], f16, kind="ExternalInput")
    b0d = nc.dram_tensor("b0", [G, 1], f32, kind="ExternalInput")
    b1d = nc.dram_tensor("b1", [G, 1], f32, kind="ExternalInput")
    Wpd = nc.dram_tensor("Wp", [2 * U, U], f16, kind="ExternalInput")
    bpd = nc.dram_tensor("bp", [U, 1], f32, kind="ExternalInput")
    skipd = nc.dram_tensor("skip", [U, ACC_W], f32, kind="ExternalInput")
    eyed = nc.dram_tensor("eye", [128, 128], f16, kind="ExternalInput")
    outd = nc.dram_tensor("out", [U, ACC_W], f32, kind="ExternalOutput")

    with ExitStack() as ctx:
        tc = ctx.enter_context(tile.TileContext(nc))
        # persistent pools
        wpool = ctx.enter_context(tc.tile_pool(name="w", bufs=1))
        big = ctx.enter_context(tc.tile_pool(name="big", bufs=1))
        state = ctx.enter_context(tc.tile_pool(name="state", bufs=3))
        dram = ctx.enter_context(tc.tile_pool(name="dram", bufs=1, space="DRAM"))

        pid = nc.partition_id()
        parity = pid % 2

        # ---- dram scratch (zx split into step-aligned chunks: 492 = 12*41)
        ZCH = 492
        NZC = 17
        zx0_d = [dram.tile([G, min(ZCH, COLS - c * ZCH)], f16, tag=f"zx0_{c}",
                           name=f"zx0_{c}")
                 for c in range(NZC)]
        zx1_d = [dram.tile([G, min(ZCH, COLS - c * ZCH)], f16, tag=f"zx1_{c}",
                           name=f"zx1_{c}")
                 for c in range(NZC)]
        h1_d = dram.tile([U, COLS], f16)
        gth_d = dram.tile([2 * U, COLS], f16)

        # ---- load + cast weights to fp16
        def load_w16(src, kt, cols, tag):
            tiles = []
            for k in range(kt):
                t16 = wpool.tile([128, cols], f16, tag=f"{tag}{k}",
                                 name=f"w_{tag}{k}")
                nc.sync.dma_start(t16[:], src[k * 128:(k + 1) * 128, :])
                tiles.append(t16)
            return tiles

        wx0 = load_w16(Wx0, KT, G, "wx0")
        wh0 = load_w16(Wh0, KT, G, "wh0")
        wx1 = load_w16(Wx1, KT, G, "wx1")
        wh1 = load_w16(Wh1, KT, G, "wh1")
        wp = load_w16(Wpd, 8, U, "wp")

        eye = wpool.tile([128, 128], f16, tag="eye")
        nc.sync.dma_start(eye[:], eyed[:])

        b0t = wpool.tile([128, MT], f32, tag="b0")
        b1t = wpool.tile([128, MT], f32, tag="b1")
        for m in range(MT):
            nc.sync.dma_start(b0t[:, m:m + 1], b0d[m * 128:(m + 1) * 128, :])
            nc.sync.dma_start(b1t[:, m:m + 1], b1d[m * 128:(m + 1) * 128, :])
        bpt = wpool.tile([128, 4], f32, tag="bp")
        for m in range(4):
            nc.sync.dma_start(bpt[:, m:m + 1], bpd[m * 128:(m + 1) * 128, :])

        # ---- load x (already fp16), k-major [128, KT*COLS]
        xh = big.tile([128, KT * COLS], f16, tag="big")
        for k in range(KT):
            nc.sync.dma_start(xh[:, k * COLS:(k + 1) * COLS],
                              xT[k * 128:(k + 1) * 128, :])

        # ---- zx GEMM: dst[g, c] = sum_u W[u, g] * rhs[u, c] + b[g]
        def zx_gemm(dst_tiles, wtiles, rhs_tile, rhs_stride, btile):
            ZCH = 492
            with tc.tile_pool(name="zxg", bufs=4) as zp, \
                 tc.tile_pool(name="zxp", bufs=6, space="PSUM") as pp:
                for c, dst in enumerate(dst_tiles):
                    c0 = c * ZCH
                    cn = dst.shape[1]
                    for m in range(MT):
                        ps = pp.tile([128, ZCH], f32, tag="ps")
                        for k in range(KT):
                            nc.tensor.matmul(
                                ps[:, :cn],
                                wtiles[k][:, m * 128:(m + 1) * 128],
                                rhs_tile[:, k * rhs_stride + c0:
                                         k * rhs_stride + c0 + cn],
                                start=(k == 0), stop=(k == KT - 1))
                        zt = zp.tile([128, ZCH], f16, tag="zt")
                        nc.scalar.activation(zt[:, :cn], ps[:, :cn],
                                             AF.Identity,
                                             bias=btile[:, m:m + 1])
                        nc.sync.dma_start(dst[m * 128:(m + 1) * 128, :cn],
                                          zt[:, :cn])

        zx_gemm(zx0_d, wx0, xh, COLS, b0t)

        # ---- LSTM recurrence
        # h layout: [128, KT*41] k-major fp16; c: [128, KT*41] f32
        def recurrence(zx_tiles, whtiles, h_sink):
            # h_sink(step, h_ap_src) emits the store of the new h
            zx_rs = [z[:].rearrange("(m p) c -> p m c", p=128)
                     for z in zx_tiles]
            with tc.tile_pool(name="rec", bufs=6) as rp, \
                 tc.tile_pool(name="recg", bufs=4) as gp, \
                 tc.tile_pool(name="rech", bufs=4) as hp, \
                 tc.tile_pool(name="recp", bufs=8, space="PSUM") as pp:
                czero = state.tile([128, KT * F], f32, tag="c")
                nc.vector.memset(czero[:], 0.0)
                hprev = hp.tile([128, KT * F], f16, tag="h")
                nc.vector.memset(hprev[:], 0.0)
                cprev = czero
                for s in range(steps):
                    zxs = rp.tile([128, MT * F], f16, tag="zxs")
                    so = (s % 12) * F
                    nc.sync.dma_start(zxs[:],
                                      zx_rs[s // 12][:, :, so:so + F])
                    gates = {}
                    for gi, gname in enumerate(("i", "f", "g", "o")):
                        ps = pp.tile([128, 4 * F], f32, tag="ps")
                        nc.tensor.matmul(ps[:], eye[:],
                                         zxs[:, gi * 4 * F:(gi + 1) * 4 * F],
                                         start=True, stop=False)
                        for jj in range(4):
                            mcol = gi * 512 + jj * 128
                            for k in range(KT):
                                nc.tensor.matmul(
                                    ps[:, jj * F:(jj + 1) * F],
                                    whtiles[k][:, mcol:mcol + 128],
                                    hprev[:, k * F:(k + 1) * F],
                                    start=False,
                                    stop=(k == KT - 1 and jj == 3),
                                    skip_group_check=True)
                        gt = gp.tile([128, 4 * F], f32, tag=f"g{gname}")
                        nc.scalar.activation(
                            gt[:], ps[:],
                            AF.Tanh if gname == "g" else AF.Sigmoid)
                        gates[gname] = gt
                    t1 = gp.tile([128, 4 * F], f32, tag="t1")
                    nc.vector.tensor_mul(t1[:], gates["i"][:], gates["g"][:])
                    cnew = state.tile([128, KT * F], f32, tag="c")
                    nc.vector.tensor_mul(cnew[:], gates["f"][:], cprev[:])
                    nc.vector.tensor_add(cnew[:], cnew[:], t1[:])
                    tcr = gp.tile([128, 4 * F], f32, tag="tc")
                    nc.scalar.activation(tcr[:], cnew[:], AF.Tanh)
                    hnew = hp.tile([128, KT * F], f16, tag="h")
                    nc.vector.tensor_mul(hnew[:], gates["o"][:], tcr[:])
                    h_sink(s, hnew)
                    hprev = hnew
                    cprev = cnew

        # f0: h goes straight into SBUF (feeds zx1 gemm)
        h0 = big.tile([128, KT * COLS], f16, tag="big")

        def sink0(s, hnew):
            nc.vector.tensor_copy(
                h0[:].rearrange("p (k c) -> p k c", k=KT)[:, :, s * F:(s + 1) * F],
                hnew[:].rearrange("p (k c) -> p k c", k=KT))

        recurrence(zx0_d, wh0, sink0)

        zx_gemm(zx1_d, wx1, h0, COLS, b1t)

        # f1: h streams to DRAM, frame-major with per-core time (un)reversal:
        # col = f*S + (parity ? S-1-s : s)
        h1_r = h1_d[:].rearrange("(k p) (f s) -> p k f s", p=128, s=S)
        sbase = parity * (S - 1)
        smul = 1 - 2 * parity

        def sink1(s, hnew):
            if do_dyn:
                off = nc.s_assert_within(sbase + s * smul, 0, S - 1,
                                         skip_runtime_assert=True)
                for k in range(KT):
                    nc.sync.dma_start(
                        h1_r[:, k, :, bass.ds(off, 1)],
                        hnew[:, k * F:(k + 1) * F])
            else:
                for k in range(KT):
                    nc.sync.dma_start(
                        h1_r[:, k, :, s:s + 1],
                        hnew[:, k * F:(k + 1) * F])

        recurrence(zx1_d, wh1, sink1)

        # ---- exchange h1 within the pair
        if do_coll:
            nc.gpsimd.collective_compute(
                "AllGather", mybir.AluOpType.bypass,
                replica_groups=[[0, 1], [2, 3], [4, 5], [6, 7]],
                ins=[h1_d[:]], outs=[gth_d[:]])
        else:
            nc.sync.dma_start(gth_d[0:U, :], h1_d[:])
            nc.sync.dma_start(gth_d[U:2 * U, :], h1_d[:])

        # ---- projection + overlap-add
        accum = big.tile([128, 4 * ACC_W], f32, tag="big")
        for k in range(4):
            nc.sync.dma_start(accum[:, k * ACC_W:(k + 1) * ACC_W],
                              skipd[k * 128:(k + 1) * 128, :])

        gth_r = gth_d[:].rearrange("(kk p) c -> p kk c", p=128)
        f0off = parity * 20 * S  # element offset into the column dim
        with tc.tile_pool(name="prj", bufs=3) as jp, \
             tc.tile_pool(name="prp", bufs=4, space="PSUM") as pp:
            for j in range(FR_LOC if do_proj else 0):
                rhs = jp.tile([128, 8 * S], f16, tag="rhs")
                nc.sync.dma_start(
                    rhs[:].rearrange("p (kk s) -> p kk s", kk=8),
                    gth_r[:, :, bass.ds(f0off + j * S, S)])
                for m in range(4):
                    ps = pp.tile([128, S], f32, tag="ps")
                    for k in range(8):
                        nc.tensor.matmul(
                            ps[:], wp[k][:, m * 128:(m + 1) * 128],
                            rhs[:, k * S:(k + 1) * S],
                            start=(k == 0), stop=(k == 7))
                    pt = jp.tile([128, S], f32, tag="pt")
                    nc.scalar.activation(pt[:], ps[:], AF.Identity,
                                         bias=bpt[:, m:m + 1])
                    a0 = j * STRIDE
                    nc.vector.tensor_add(
                        accum[:, m * ACC_W + a0:m * ACC_W + a0 + S],
                        accum[:, m * ACC_W + a0:m * ACC_W + a0 + S],
                        pt[:])

        # ---- output
        for k in range(4):
            nc.sync.dma_start(outd[k * 128:(k + 1) * 128, :],
                              accum[:, k * ACC_W:(k + 1) * ACC_W])

    nc.compile()
    return nc


def _prep_inputs(inputs, Wx_f0, Wh_f0, b_f0, Wx_f1, Wh_f1, b_f1,
                 Wx_b0, Wh_b0, b_b0, Wx_b1, Wh_b1, b_b1, Wp, bp):
    x = np.asarray(inputs, dtype=np.float32)  # [4, 512, 4200]
    eye = np.eye(128, dtype=np.float16)
    idx = np.arange(F)[:, None] * STRIDE + np.arange(S)[None, :]  # [F, S]
    wsets = {
        0: (Wx_f0, Wh_f0, b_f0, Wx_f1, Wh_f1, b_f1),
        1: (Wx_b0, Wh_b0, b_b0, Wx_b1, Wh_b1, b_b1),
    }
    in_maps = []
    for c in range(NCORES):
        q, parity = c // 2, c % 2
        xs = x[q][:, idx]                       # [U, F, S]
        if parity:
            xs = xs[:, :, ::-1]
        xTc = np.ascontiguousarray(
            xs.transpose(0, 2, 1).reshape(U, COLS)).astype(np.float16)
        wx0, wh0, b0, wx1, wh1, b1 = wsets[parity]
        # skip goes only into the kept window (pads are trimmed by the host)
        sk = np.zeros((U, ACC_W), dtype=np.float32)
        if parity == 0:
            sk[:, 0:SEG] = x[q][:, 0:SEG]          # kept window [0:2100)
        else:
            sk[:, PAD:PAD + SEG] = x[q][:, SEG:T]  # kept window [100:2200)
        in_maps.append({
            "xT": xTc,
            "Wx0": np.asarray(wx0, np.float16),
            "Wh0": np.asarray(wh0, np.float16),
            "Wx1": np.asarray(wx1, np.float16),
            "Wh1": np.asarray(wh1, np.float16),
            "b0": np.asarray(b0, np.float32).reshape(G, 1),
            "b1": np.asarray(b1, np.float32).reshape(G, 1),
            "Wp": np.asarray(Wp, np.float16),
            "bp": np.asarray(bp, np.float32).reshape(U, 1),
            "skip": sk,
            "eye": eye,
        })
    return in_maps


def kernel(**inputs) -> np.ndarray:
    from concourse.bass_utils import run_bass_kernel_spmd

    if "nc" not in _CACHE:
        _CACHE["nc"] = _build()
    nc = _CACHE["nc"]

    import os
    in_maps = _prep_inputs(**inputs)
    trace = os.environ.get("BL_TRACE", "0") == "1"
    res = run_bass_kernel_spmd(nc, in_maps, list(range(NCORES)), trace=trace)
    _CACHE["last_result"] = res

    out = np.zeros((4, U, T), dtype=np.float32)
    for c in range(NCORES):
        q, parity = c // 2, c % 2
        seg = res.results[c]["out"]  # [U, ACC_W]
        if parity == 0:
            out[q][:, 0:SEG] = seg[:, 0:SEG]
        else:
            out[q][:, SEG:T] = seg[:, PAD:PAD + SEG]
    return out



# revision 7
# speedup vs baseline: 2.4358x; 2.4358x over previous
"""BLSTM Trainium2 kernel: 8-core SPMD, wavefront schedule.

Core pair q={2q,2q+1} owns batch element q. Even core runs the forward
2-layer LSTM chain, odd core the backward chain (host feeds frames with
both the step axis and the frame order reversed, which makes the device
program parity-free). Per 8-step chunk, the schedule interleaves on one
PE queue: layer-0 recurrence, layer-1 recurrence (1 chunk behind),
the zx input GEMMs for both layers, and the per-stream projection with
overlap-add into a dual accumulator (natural lower half + reversed
upper half). A single pairwise ReduceScatter(add) at the end combines
the two streams; the host flips the odd core's segment.
"""
import numpy as np
from contextlib import ExitStack

U = 512
S = 200          # frame width (LSTM steps)
F = 41           # frames per batch element
T = 4200
STRIDE = 100
HALF = 2100
COLS = S * F     # 8200 device columns, col = s*41 + f
G = 4 * U        # 2048 gate rows
NCORES = 8
KT = U // 128    # 4 k-tiles
MT = G // 128    # 16 m-tiles
CH = 8           # steps per chunk
NCH = S // CH    # 25 chunks

_CACHE = {}


def _build():
    import os
    do_coll = os.environ.get("BL_COLLECTIVE", "1") == "1"
    import concourse.bacc as bacc
    import concourse.tile as tile
    import concourse.bass as bass
    from concourse import mybir
    from concourse.alu_op_type import AluOpType

    f32 = mybir.dt.float32
    f16 = mybir.dt.float16
    AF = mybir.ActivationFunctionType

    nc = bacc.Bacc("TRN2", target_bir_lowering=False, debug=False,
                   num_devices=NCORES)

    xT = nc.dram_tensor("xT", [U, COLS], f16, kind="ExternalInput")
    Wx0 = nc.dram_tensor("Wx0", [U, G], f16, kind="ExternalInput")
    Wh0 = nc.dram_tensor("Wh0", [U, G], f16, kind="ExternalInput")
    Wx1 = nc.dram_tensor("Wx1", [U, G], f16, kind="ExternalInput")
    Wh1 = nc.dram_tensor("Wh1", [U, G], f16, kind="ExternalInput")
    b0d = nc.dram_tensor("b0", [G, 1], f32, kind="ExternalInput")
    b1d = nc.dram_tensor("b1", [G, 1], f32, kind="ExternalInput")
    Wpd = nc.dram_tensor("Wp", [U, U], f16, kind="ExternalInput")
    bpd = nc.dram_tensor("bp", [U, 1], f32, kind="ExternalInput")
    skipd = nc.dram_tensor("skip", [U, HALF], f32, kind="ExternalInput")
    eyed = nc.dram_tensor("eye", [128, 128], f16, kind="ExternalInput")
    outd = nc.dram_tensor("out", [U, HALF], f32, kind="ExternalOutput")

    with ExitStack() as ctx:
        tc = ctx.enter_context(tile.TileContext(nc))
        wpool = ctx.enter_context(tc.tile_pool(name="w", bufs=1))
        accp = ctx.enter_context(tc.tile_pool(name="acc", bufs=1))
        xp = ctx.enter_context(tc.tile_pool(name="x", bufs=2))
        zp = [ctx.enter_context(tc.tile_pool(name=f"z{l}", bufs=2))
              for l in range(2)]
        hp = [ctx.enter_context(tc.tile_pool(name=f"h{l}", bufs=2))
              for l in range(2)]
        gp = [ctx.enter_context(tc.tile_pool(name=f"g{l}", bufs=1))
              for l in range(2)]
        cpools = [ctx.enter_context(tc.tile_pool(name=f"c{l}", bufs=2))
                  for l in range(2)]
        tp = [ctx.enter_context(tc.tile_pool(name=f"t{l}", bufs=1))
              for l in range(2)]
        ptp = ctx.enter_context(tc.tile_pool(name="pt", bufs=2))
        psr = [ctx.enter_context(
            tc.tile_pool(name=f"psr{l}", bufs=1, space="PSUM"))
            for l in range(2)]
        psg = ctx.enter_context(tc.tile_pool(name="psg", bufs=2, space="PSUM"))
        psp = ctx.enter_context(tc.tile_pool(name="psp", bufs=2, space="PSUM"))
        dram = ctx.enter_context(tc.tile_pool(name="dram", bufs=1,
                                              space="DRAM"))

        in_d = dram.tile([2 * U, HALF], f32, name="in_d")
        rs_d = dram.tile([U, HALF], f32, name="rs_d")

        # ---- weights / constants
        def load_w(src, tag, cols):
            tiles = []
            for k in range(KT):
                t = wpool.tile([128, cols], f16, tag=f"{tag}{k}",
                               name=f"w_{tag}{k}")
                nc.sync.dma_start(t[:], src[k * 128:(k + 1) * 128, :])
                tiles.append(t)
            return tiles

        wx = [load_w(Wx0, "wx0", G), load_w(Wx1, "wx1", G)]
        wh = [load_w(Wh0, "wh0", G), load_w(Wh1, "wh1", G)]
        wp = load_w(Wpd, "wp", U)

        eye = wpool.tile([128, 128], f16, tag="eye")
        nc.sync.dma_start(eye[:], eyed[:])

        b0t = wpool.tile([128, MT], f32, tag="b0")
        nc.sync.dma_start(b0t[:], b0d[:].rearrange("(m p) o -> p (m o)", p=128))
        b1t = wpool.tile([128, MT], f32, tag="b1")
        nc.sync.dma_start(b1t[:], b1d[:].rearrange("(m p) o -> p (m o)", p=128))
        bt = [b0t, b1t]
        bpt = wpool.tile([128, KT], f32, tag="bp")
        nc.sync.dma_start(bpt[:], bpd[:].rearrange("(m p) o -> p (m o)", p=128))

        # ---- accumulators: accA = skip-initialized lower half (natural),
        # accB = upper half in reversed device time
        accA = accp.tile([128, KT, HALF], f32, tag="accA")
        nc.sync.dma_start(accA[:], skipd[:].rearrange("(k p) c -> p k c", p=128))
        accB = accp.tile([128, KT, HALF], f32, tag="accB")
        nc.vector.memset(accB[:, 0:2, :], 0.0)
        nc.gpsimd.memset(accB[:, 2:4, :], 0.0)
        accBr = accB[:, :, ::-1]

        xr = xT[:].rearrange("(k p) c -> p k c", p=128)
        xt = [None] * NCH
        zt = [[None] * NCH for _ in range(2)]
        ht = [[None] * NCH for _ in range(2)]
        cst = [None, None]

        def emit_X(c):
            xt[c] = xp.tile([128, KT, CH, F], f16, tag="x", name=f"x{c}")
            nc.sync.dma_start(
                xt[c][:].rearrange("p k s f -> p k (s f)"),
                xr[:, :, c * CH * F:(c + 1) * CH * F])

        def emit_G(l, c, m_lo, m_hi):
            # zx GEMM for layer l, chunk c, m-tiles [m_lo, m_hi)
            src = xt[c] if l == 0 else ht[0][c]
            if m_lo == 0:
                zt[l][c] = zp[l].tile([128, MT, CH, F], f16, tag="z", name=f"z{l}_{c}")
            z = zt[l][c]
            for m in range(m_lo, m_hi):
                ps = psg.tile([128, CH * F], f32, tag="ps")
                for k in range(KT):
                    nc.tensor.matmul(ps[:], wx[l][k][:, m * 128:(m + 1) * 128],
                                     src[:, k, :, :],
                                     start=(k == 0), stop=(k == KT - 1))
                nc.scalar.activation(z[:, m, :, :], ps[:], AF.Identity,
                                     bias=bt[l][:, m:m + 1])

        def emit_rec_step(l, s):
            c, si = divmod(s, CH)
            if si == 0:
                ht[l][c] = hp[l].tile([128, KT, CH, F], f16, tag="h", name=f"h{l}_{c}")
            z = zt[l][c]
            ps_if = psr[l].tile([128, 8 * F], f32, tag="if")
            ps_go = psr[l].tile([128, 8 * F], f32, tag="go")
            nc.tensor.matmul(ps_if[:], eye[:], z[:, 0:8, si, :],
                             start=True, stop=(s == 0))
            nc.tensor.matmul(ps_go[:], eye[:], z[:, 8:16, si, :],
                             start=True, stop=(s == 0))
            if s > 0:
                hc, hsi = ((ht[l][c - 1], CH - 1) if si == 0
                           else (ht[l][c], si - 1))
                for ps, m_lo in ((ps_if, 0), (ps_go, 8)):
                    for mi in range(8):
                        m = m_lo + mi
                        for k in range(KT):
                            nc.tensor.matmul(
                                ps[:, mi * F:(mi + 1) * F],
                                wh[l][k][:, m * 128:(m + 1) * 128],
                                hc[:, k, hsi, :],
                                start=False,
                                stop=(mi == 7 and k == KT - 1),
                                skip_group_check=True)
            sif = gp[l].tile([128, 8 * F], f32, tag="sif")
            nc.scalar.activation(sif[:], ps_if[:], AF.Sigmoid)
            sgo = gp[l].tile([128, 8 * F], f32, tag="sgo")
            nc.scalar.activation(sgo[:, 0:4 * F], ps_go[:, 0:4 * F], AF.Tanh)
            nc.scalar.activation(sgo[:, 4 * F:], ps_go[:, 4 * F:], AF.Sigmoid)
            cnew = cpools[l].tile([128, 4 * F], f32, tag="c")
            if s == 0:
                nc.vector.tensor_mul(cnew[:], sif[:, 0:4 * F], sgo[:, 0:4 * F])
            else:
                t1 = tp[l].tile([128, 4 * F], f32, tag="t1")
                nc.vector.tensor_mul(t1[:], sif[:, 0:4 * F], sgo[:, 0:4 * F])
                t2 = tp[l].tile([128, 4 * F], f32, tag="t2")
                nc.gpsimd.tensor_mul(t2[:], sif[:, 4 * F:], cst[l][:])
                nc.vector.tensor_add(cnew[:], t1[:], t2[:])
            th = tp[l].tile([128, 4 * F], f32, tag="th")
            nc.scalar.activation(th[:], cnew[:], AF.Tanh)
            nc.gpsimd.tensor_mul(ht[l][c][:, :, si, :], sgo[:, 4 * F:], th[:])
            cst[l] = cnew

        def emit_P(c):
            pt = ptp.tile([128, KT, CH, F], f16, tag="pt")
            for m in range(KT):
                ps = psp.tile([128, CH * F], f32, tag="ps")
                for k in range(KT):
                    nc.tensor.matmul(ps[:], wp[k][:, m * 128:(m + 1) * 128],
                                     ht[1][c][:, k, :, :],
                                     start=(k == 0), stop=(k == KT - 1))
                nc.scalar.activation(pt[:, m, :, :], ps[:], AF.Identity,
                                     bias=bpt[:, m:m + 1])
            for si in range(CH):
                sg = c * CH + si
                cntA = 21 if sg < 100 else 20
                endA = sg + (cntA - 1) * 100 + 1
                nc.vector.tensor_add(accA[:, :, sg:endA:100],
                                     accA[:, :, sg:endA:100],
                                     pt[:, :, si, 0:cntA])
                cntB = F - cntA
                base = cntA * 100 + sg - HALF
                endB = base + (cntB - 1) * 100 + 1
                nc.gpsimd.tensor_add(accBr[:, :, base:endB:100],
                                     accBr[:, :, base:endB:100],
                                     pt[:, :, si, cntA:F])

        # ---- wavefront
        emit_X(0)
        emit_X(1)
        emit_G(0, 0, 0, MT)
        for c in range(NCH):
            if c + 2 < NCH:
                emit_X(c + 2)
            for si in range(CH):
                emit_rec_step(0, c * CH + si)
                if c >= 1:
                    emit_rec_step(1, (c - 1) * CH + si)
                if c + 1 < NCH:
                    emit_G(0, c + 1, 2 * si, 2 * si + 2)
            emit_G(1, c, 0, MT)
            if c >= 1:
                emit_P(c - 1)
        for si in range(CH):
            emit_rec_step(1, (NCH - 1) * CH + si)
        emit_P(NCH - 1)

        # ---- pairwise exchange: my rank's block gets accA (my half),
        # partner's block gets accB (their half, already in their coords)
        pid = nc.partition_id()
        rank = nc.s_assert_within(pid % 2, 0, 1, skip_runtime_assert=True)
        other = nc.s_assert_within(1 - pid % 2, 0, 1, skip_runtime_assert=True)
        in_r = in_d[:].rearrange("(b k p) c -> p b k c", p=128, k=KT)
        nc.sync.dma_start(in_r[:, bass.ds(rank, 1)], accA[:])
        nc.sync.dma_start(in_r[:, bass.ds(other, 1)], accB[:])
        if do_coll:
            nc.gpsimd.collective_compute(
                "ReduceScatter", AluOpType.add,
                replica_groups=[[0, 1], [2, 3], [4, 5], [6, 7]],
                ins=[in_d[:]], outs=[rs_d[:]])
        else:
            nc.sync.dma_start(rs_d[:], in_d[U:2 * U, :])
        nc.sync.dma_start(outd[:], rs_d[:])

    nc.compile()
    return nc


def _prep_inputs(inputs, Wx_f0, Wh_f0, b_f0, Wx_f1, Wh_f1, b_f1,
                 Wx_b0, Wh_b0, b_b0, Wx_b1, Wh_b1, b_b1, Wp, bp):
    x = np.asarray(inputs, dtype=np.float32)  # [4, 512, 4200]
    eye = np.eye(128, dtype=np.float16)
    idx = np.arange(F)[:, None] * STRIDE + np.arange(S)[None, :]  # [F, S]
    wsets = {
        0: (Wx_f0, Wh_f0, b_f0, Wx_f1, Wh_f1, b_f1),
        1: (Wx_b0, Wh_b0, b_b0, Wx_b1, Wh_b1, b_b1),
    }
    Wp = np.asarray(Wp)
    bph = (np.asarray(bp, np.float32) * 0.5).reshape(U, 1)
    in_maps = []
    for core in range(NCORES):
        q, par = core // 2, core % 2
        xs = x[q][:, idx]                       # [U, F, S]
        if par:
            xs = xs[:, ::-1, ::-1]
            skip = np.ascontiguousarray(x[q][:, HALF:][:, ::-1])
            Wp_own = Wp[U:]
        else:
            skip = np.ascontiguousarray(x[q][:, :HALF])
            Wp_own = Wp[:U]
        xdev = np.ascontiguousarray(
            xs.transpose(0, 2, 1).reshape(U, COLS)).astype(np.float16)
        wx0, wh0, b0, wx1, wh1, b1 = wsets[par]
        in_maps.append({
            "xT": xdev,
            "Wx0": np.asarray(wx0, np.float16),
            "Wh0": np.asarray(wh0, np.float16),
            "Wx1": np.asarray(wx1, np.float16),
            "Wh1": np.asarray(wh1, np.float16),
            "b0": np.asarray(b0, np.float32).reshape(G, 1),
            "b1": np.asarray(b1, np.float32).reshape(G, 1),
            "Wp": np.asarray(Wp_own, np.float16),
            "bp": bph,
            "skip": skip,
            "eye": eye,
        })
    return in_maps


def kernel(**inputs) -> np.ndarray:
    from concourse.bass_utils import run_bass_kernel_spmd

    if "nc" not in _CACHE:
        _CACHE["nc"] = _build()
    nc = _CACHE["nc"]

    import os
    in_maps = _prep_inputs(**inputs)
    trace = os.environ.get("BL_TRACE", "0") == "1"
    res = run_bass_kernel_spmd(nc, in_maps, list(range(NCORES)), trace=trace)
    _CACHE["last_result"] = res

    out = np.zeros((4, U, T), dtype=np.float32)
    for core in range(NCORES):
        q, par = core // 2, core % 2
        seg = res.results[core]["out"]  # [U, HALF]
        if par == 0:
            out[q][:, :HALF] = seg
        else:
            out[q][:, HALF:] = seg[:, ::-1]
    return out
